# revision 11
# baseline (speedup 1.0000x reference)
"""Trainium2 Bass kernel for nn_KeyDecider: per-(b,ch) spatial softmax +
soft-argmax + confidence, batch-sharded across 8 NeuronCores.

Input : x [64, 34, 256, 256] f32     Output: [64, 17, 3] f32 (co_x, co_y, vi)

Math per (b, c<17): w = softmax(x[b,c].ravel()); v = x[b,c+17].ravel()
  ki = round(sum(w*p));  out = [ki%256, ki//256, sum(w*v)]
exp() needs no max-subtraction here (randn inputs, |x|<6): one pass over HBM.

Device strategy (per core, 8 local batches; HBM-bandwidth bound):
- DMA 4-channel contiguous slabs of one batch as flat [128, 2048] f32 tiles.
  Flat 2-dim APs spread descriptors over all 16 SDMA engines (3-dim APs only
  reach 8 of them - measured), so this hits the ~358 GB/s per-core HBM limit.
  Partition p = (channel cc=p//32, spatial block s32=p%32), free l in [0,2048).
- ACT: e = exp(h) with fused accum_out -> exact f32 per-partition rowsums r
  (the precision-critical soft-argmax term T1 derives from these).
- PE : per-channel colsums via "tagged" matmuls: stationary [128,128]
  indicator routes partition p to PSUM row tag=b*16+c, accumulating
  CE[tag,l] = sum_p e[p,l] across all quads in fp32r (full-rate moving
  operand, ~1e-4 relative accuracy - enough for the T2 term since l<2048).
- DVE: m = e*v (bf16 out); per-tile rowsums of m (split with ACT) give s2;
  final dots T2[tag] = sum_l l*CE[tag,l] on PSUM.
- Host (f64): s0 = sum r; T1 = sum 2048*s32*r; s1 = T1+T2;
  ki = round(s1/s0); co_x = ki%256, co_y = ki//256, vi = s2/s0.
"""

import sys

for _p in ("/opt/trn_rl_repo", "/root/.axon_site/_ro/trn_rl_repo"):
    if _p not in sys.path:
        sys.path.insert(0, _p)

import numpy as np

B, C, K = 64, 34, 17
W = H = 256
N = W * H                  # 65536
IMG_W = IMG_H = 256.0
NCORES = 8
BPC = B // NCORES          # 8 batches per core
NQ = 4                     # channel quads (channels 0-15); c=16 handled alone
QW = 4 * N // 128          # 2048 free elems per partition in a quad tile
SW = N // 128              # 512 free elems in a single-channel tile
NSTAT = BPC * 5 + BPC * 5  # 4+1 exp-rowsum cols per batch, then 4+1 m-rowsum

_cache = {}


def _build(reps: int = 1):
    import concourse.bass as bass
    import concourse.bacc as bacc
    import concourse.tile as tile
    from concourse import mybir

    f32 = mybir.dt.float32
    f32r = mybir.dt.float32r
    f16 = mybir.dt.float16
    bf16 = mybir.dt.bfloat16
    Exp = mybir.ActivationFunctionType.Exp
    Ident = mybir.ActivationFunctionType.Identity

    nc = bacc.Bacc("TRN2", target_bir_lowering=False, debug=False)
    x_d = nc.declare_dram_parameter("x", [BPC, K, N], f32r, isOutput=False)
    xv_d = nc.declare_dram_parameter("xv", [BPC, K, N], f16, isOutput=False)
    # sliceable stationary masters (host-filled indicator matrices):
    # mq_r[p, 128 + p//32] = 1  ->  slice [128-tb : 256-tb] maps p to
    # column tb + p//32 (tb = b*16+4q), i.e. PSUM row b*16+c.
    mqr_d = nc.declare_dram_parameter("mq_r", [128, 256], f32r, isOutput=False)
    msr_d = nc.declare_dram_parameter("ms_r", [128, 256], f32r, isOutput=False)
    stats_d = nc.declare_dram_parameter("stats", [128, NSTAT], f32, isOutput=True)
    aux_d = nc.declare_dram_parameter("aux", [128, 4], f32, isOutput=True)
    x_ap = x_d[:]
    xv_ap = xv_d[:]

    with tile.TileContext(nc) as tc:
        with (
            tc.tile_pool(name="hp", bufs=7) as hp,
            tc.tile_pool(name="vp", bufs=8) as vp,
            tc.tile_pool(name="ep", bufs=3) as ep,
            tc.tile_pool(name="mp", bufs=4) as mp,
            tc.tile_pool(name="cst", bufs=1) as cst,
            tc.tile_pool(name="st", bufs=1) as st,
            tc.tile_pool(name="ps", bufs=1, space="PSUM") as pp,
            tc.tile_pool(name="dr", bufs=1) as dr,
        ):
            mq_r = cst.tile([128, 256], f32r)
            nc.sync.dma_start(out=mq_r[:], in_=mqr_d[:])
            ms_r = cst.tile([128, 256], f32r)
            nc.sync.dma_start(out=ms_r[:], in_=msr_d[:])

            li = cst.tile([128, 2048], mybir.dt.int32)
            nc.gpsimd.iota(li[:], pattern=[[1, 2048]], base=0, channel_multiplier=0)
            lf = cst.tile([128, 2048], f32)
            nc.vector.tensor_copy(lf[:], li[:])

            stats_t = st.tile([128, NSTAT], f32)
            aux_t = st.tile([128, 4], f32)

            for _ in range(reps):
                psE = pp.tile([128, 2048], f32)
                psEs = pp.tile([128, 512], f32)
                touched: set = set()

                for b in range(BPC):
                    for q in range(NQ):
                        src_h = bass.AP(
                            tensor=x_ap.tensor,
                            offset=b * K * N + (4 * q) * N,
                            ap=[[QW, 128], [1, QW]],
                        )
                        ht = hp.tile([128, QW], f32r)
                        nc.sync.dma_start(out=ht[:], in_=src_h)
                        src_v = bass.AP(
                            tensor=xv_ap.tensor,
                            offset=b * K * N + (4 * q) * N,
                            ap=[[QW, 128], [1, QW]],
                        )
                        vt = vp.tile([128, QW], f16)
                        nc.sync.dma_start(out=vt[:], in_=src_v)

                        et = ep.tile([128, QW], f32r)
                        nc.scalar.activation(
                            et[:], ht[:], Exp,
                            accum_out=stats_t[:, b * 5 + q:b * 5 + q + 1],
                        )
                        mt = mp.tile([128, QW], bf16)
                        if (b * NQ + q) % 5 == 2:
                            nc.gpsimd.tensor_mul(mt[:], et[:], vt[:])
                        else:
                            nc.vector.tensor_tensor(
                                out=mt[:], in0=et[:], in1=vt[:],
                                op=mybir.AluOpType.mult,
                            )

                        tb = b * 16 + 4 * q
                        statr = mq_r[:, 128 - tb:256 - tb]
                        last = b == BPC - 1 and q == NQ - 1
                        for j in range(4):
                            nc.tensor.matmul(
                                out=psE[:, j * 512:(j + 1) * 512],
                                lhsT=statr, rhs=et[:, j * 512:(j + 1) * 512],
                                start=j not in touched, stop=last,
                            )
                            touched.add(j)
                        mcol = BPC * 5 + b * 5 + q
                        if (b + q) % 2 == 0:
                            nc.scalar.activation(
                                mt[:], mt[:], Ident,
                                accum_out=stats_t[:, mcol:mcol + 1],
                            )
                        else:
                            nc.vector.reduce_sum(
                                stats_t[:, mcol:mcol + 1], mt[:],
                                axis=mybir.AxisListType.X,
                            )

                # single channels c=16 (heatmap) / c=33 (uncertainty) first:
                # their small serial PSUM chain completes early, hidden
                # under the DMA ramp; the end of the stream is then pure
                # deep-pipelined quad work.
                for b in range(BPC):
                    src_hs = bass.AP(
                        tensor=x_ap.tensor,
                        offset=b * K * N + 16 * N,
                        ap=[[SW, 128], [1, SW]],
                    )
                    hs = hp.tile([128, SW], f32r, tag="hs")
                    nc.sync.dma_start(out=hs[:], in_=src_hs)
                    src_vs = bass.AP(
                        tensor=xv_ap.tensor,
                        offset=b * K * N + 16 * N,
                        ap=[[SW, 128], [1, SW]],
                    )
                    vs = vp.tile([128, SW], f16, tag="vs")
                    nc.sync.dma_start(out=vs[:], in_=src_vs)

                    es = ep.tile([128, SW], f32r, tag="es")
                    nc.scalar.activation(
                        es[:], hs[:], Exp,
                        accum_out=stats_t[:, b * 5 + 4:b * 5 + 5],
                    )
                    msi = mp.tile([128, SW], bf16, tag="msi")
                    nc.vector.tensor_tensor(
                        out=msi[:], in0=es[:], in1=vs[:], op=mybir.AluOpType.mult,
                    )
                    stats_s = ms_r[:, 128 - b:256 - b]
                    nc.tensor.matmul(out=psEs[:, :], lhsT=stats_s, rhs=es[:],
                                     start=(b == 0), stop=(b == BPC - 1))
                    mscol = BPC * 5 + b * 5 + 4
                    nc.vector.reduce_sum(
                        stats_t[:, mscol:mscol + 1], msi[:],
                        axis=mybir.AxisListType.X,
                    )

                # T2 dots against the position ramp, straight off PSUM
                cel = dr.tile([128, 2048], f32)
                nc.vector.tensor_tensor(out=cel[:], in0=psE[:], in1=lf[:],
                                        op=mybir.AluOpType.mult)
                nc.vector.reduce_sum(aux_t[:, 0:1], cel[:],
                                     axis=mybir.AxisListType.X)
                ces = dr.tile([8, 512], f32, tag="ces")
                nc.vector.tensor_tensor(out=ces[:], in0=psEs[0:8, :],
                                        in1=lf[0:8, 0:512],
                                        op=mybir.AluOpType.mult)
                nc.vector.reduce_sum(aux_t[0:8, 2:3], ces[:],
                                     axis=mybir.AxisListType.X)

            nc.sync.dma_start(out=stats_d[:], in_=stats_t[:])
            nc.sync.dma_start(out=aux_d[:], in_=aux_t[:])

    nc.compile()
    return nc


def _masters():
    mq = np.zeros((128, 256), np.float32)
    for p in range(128):
        mq[p, 128 + p // 32] = 1.0
    ms = np.zeros((128, 256), np.float32)
    ms[:, 128] = 1.0
    return {"mq_r": mq, "ms_r": ms}


def _split_inputs(x: np.ndarray):
    """Per-core h (f32, channels 0..16) and v (fp16, channels 17..33).
    fp16 v halves the v-side HBM traffic; vi tolerates it easily.
    astype/copy release the GIL, so thread across cores and halves."""
    from concurrent.futures import ThreadPoolExecutor

    xr = x.reshape(B, C, N)

    def mk(i):
        sl = xr[i * BPC:(i + 1) * BPC]
        return (np.ascontiguousarray(sl[:, :K]),
                sl[:, K:].astype(np.float16))

    with ThreadPoolExecutor(max_workers=NCORES) as ex:
        return list(ex.map(mk, range(NCORES)))


def _run_device(x: np.ndarray, reps: int = 1, trace: bool = False,
                tmpdir: str | None = None):
    from concourse.bass_utils import run_bass_kernel_spmd

    if reps not in _cache:
        _cache[reps] = _build(reps)
    nc = _cache[reps]
    m = _masters()
    in_maps = [
        {"x": xh, "xv": xv, **m}
        for xh, xv in _split_inputs(x)
    ]
    return run_bass_kernel_spmd(nc, in_maps, list(range(NCORES)),
                                trace=trace, tmpdir=tmpdir)


def _finish(results) -> np.ndarray:
    out = np.empty((B, K, 3), np.float32)
    s32 = np.arange(32, dtype=np.float64)    # spatial block within a quad row
    p128 = np.arange(128, dtype=np.float64)  # spatial block for singles
    for i in range(NCORES):
        r = results[i]
        stats = r["stats"].astype(np.float64)   # [128, 80]
        aux = r["aux"].astype(np.float64)       # [128, 4]
        for b in range(BPC):
            for c in range(K):
                if c < 16:
                    q, cc = c // 4, c % 4
                    rv = stats[cc * 32:(cc + 1) * 32, b * 5 + q]  # [32]
                    s0 = rv.sum()
                    t1 = (2048.0 * s32 * rv).sum()
                    t2 = aux[b * 16 + c, 0]
                    s2 = stats[cc * 32:(cc + 1) * 32, BPC * 5 + b * 5 + q].sum()
                else:
                    rv = stats[:, b * 5 + 4]
                    s0 = rv.sum()
                    t1 = (512.0 * p128 * rv).sum()
                    t2 = aux[b, 2]
                    s2 = stats[:, BPC * 5 + b * 5 + 4].sum()
                ki = np.round((t1 + t2) / s0)
                out[i * BPC + b, c, 0] = np.mod(ki, W) / W * IMG_W
                out[i * BPC + b, c, 1] = np.floor(ki / W) / H * IMG_H
                out[i * BPC + b, c, 2] = s2 / s0
    return out


def kernel(x: np.ndarray) -> np.ndarray:
    res = _run_device(np.asarray(x), reps=1)
    return _finish(res.results)


# revision 12
# speedup vs baseline: 1.0838x; 1.0838x over previous
"""Trainium2 Bass kernel for nn_KeyDecider: per-(b,ch) spatial softmax +
soft-argmax + confidence, batch-sharded across 8 NeuronCores.

Input : x [64, 34, 256, 256] f32     Output: [64, 17, 3] f32 (co_x, co_y, vi)

Math per (b, c<17): w = softmax(x[b,c].ravel()); v = x[b,c+17].ravel()
  ki = round(sum(w*p));  out = [ki%256, ki//256, sum(w*v)]
exp() needs no max-subtraction here (randn inputs, |x|<6): one pass over HBM.

Device strategy (per core, 8 local batches; HBM-bandwidth bound):
- DMA 4-channel contiguous slabs of one batch as flat [128, 2048] f32 tiles.
  Flat 2-dim APs spread descriptors over all 16 SDMA engines (3-dim APs only
  reach 8 of them - measured), so this hits the ~358 GB/s per-core HBM limit.
  Partition p = (channel cc=p//32, spatial block s32=p%32), free l in [0,2048).
- ACT: e = exp(h) with fused accum_out -> exact f32 per-partition rowsums r
  (the precision-critical soft-argmax term T1 derives from these).
- PE : per-channel colsums via "tagged" matmuls: stationary [128,128]
  indicator routes partition p to PSUM row tag=b*16+c, accumulating
  CE[tag,l] = sum_p e[p,l] across all quads in fp32r (full-rate moving
  operand, ~1e-4 relative accuracy - enough for the T2 term since l<2048).
- DVE: m = e*v (bf16 out); per-tile rowsums of m (split with ACT) give s2;
  final dots T2[tag] = sum_l l*CE[tag,l] on PSUM.
- Host (f64): s0 = sum r; T1 = sum 2048*s32*r; s1 = T1+T2;
  ki = round(s1/s0); co_x = ki%256, co_y = ki//256, vi = s2/s0.
"""

import sys

for _p in ("/opt/trn_rl_repo", "/root/.axon_site/_ro/trn_rl_repo"):
    if _p not in sys.path:
        sys.path.insert(0, _p)

import numpy as np

B, C, K = 64, 34, 17
W = H = 256
N = W * H                  # 65536
IMG_W = IMG_H = 256.0
NCORES = 8
BPC = B // NCORES          # 8 batches per core
NQ = 4                     # channel quads (channels 0-15); c=16 handled alone
QW = 4 * N // 128          # 2048 free elems per partition in a quad tile
SW = N // 128              # 512 free elems in a single-channel tile
NSTAT = BPC * 5 + BPC * 5  # 4+1 exp-rowsum cols per batch, then 4+1 m-rowsum

_cache = {}


def _build(reps: int = 1):
    import concourse.bass as bass
    import concourse.bacc as bacc
    import concourse.tile as tile
    from concourse import mybir

    f32 = mybir.dt.float32
    f32r = mybir.dt.float32r
    f16 = mybir.dt.float16
    bf16 = mybir.dt.bfloat16
    Exp = mybir.ActivationFunctionType.Exp
    Ident = mybir.ActivationFunctionType.Identity

    nc = bacc.Bacc("TRN2", target_bir_lowering=False, debug=False)
    x_d = nc.declare_dram_parameter("x", [BPC, K, N], f32r, isOutput=False)
    xv_d = nc.declare_dram_parameter("xv", [BPC, K, N], f16, isOutput=False)
    # sliceable stationary masters (host-filled indicator matrices):
    # mq_r[p, 128 + p//32] = 1  ->  slice [128-tb : 256-tb] maps p to
    # column tb + p//32 (tb = b*16+4q), i.e. PSUM row b*16+c.
    mqr_d = nc.declare_dram_parameter("mq_r", [128, 256], f32r, isOutput=False)
    msr_d = nc.declare_dram_parameter("ms_r", [128, 256], f32r, isOutput=False)
    stats_d = nc.declare_dram_parameter("stats", [128, NSTAT], f32, isOutput=True)
    aux_d = nc.declare_dram_parameter("aux", [128, 4], f32, isOutput=True)
    x_ap = x_d[:]
    xv_ap = xv_d[:]

    with tile.TileContext(nc) as tc:
        with (
            tc.tile_pool(name="hp", bufs=7) as hp,
            tc.tile_pool(name="vp", bufs=8) as vp,
            tc.tile_pool(name="ep", bufs=3) as ep,
            tc.tile_pool(name="mp", bufs=4) as mp,
            tc.tile_pool(name="cst", bufs=1) as cst,
            tc.tile_pool(name="st", bufs=1) as st,
            tc.tile_pool(name="ps", bufs=1, space="PSUM") as pp,
            tc.tile_pool(name="dr", bufs=1) as dr,
        ):
            mq_r = cst.tile([128, 256], f32r)
            nc.sync.dma_start(out=mq_r[:], in_=mqr_d[:])
            ms_r = cst.tile([128, 256], f32r)
            nc.sync.dma_start(out=ms_r[:], in_=msr_d[:])

            li = cst.tile([128, 2048], mybir.dt.int32)
            nc.gpsimd.iota(li[:], pattern=[[1, 2048]], base=0, channel_multiplier=0)
            lf = cst.tile([128, 2048], f32)
            nc.vector.tensor_copy(lf[:], li[:])

            stats_t = st.tile([128, NSTAT], f32)
            aux_t = st.tile([128, 4], f32)

            for _ in range(reps):
                psE = pp.tile([128, 2048], f32)
                psEs = pp.tile([128, 512], f32)
                touched: set = set()

                for b in range(BPC):
                    for q in range(NQ):
                        src_h = bass.AP(
                            tensor=x_ap.tensor,
                            offset=b * K * N + (4 * q) * N,
                            ap=[[QW, 128], [1, QW]],
                        )
                        ht = hp.tile([128, QW], f32r)
                        nc.sync.dma_start(out=ht[:], in_=src_h)
                        src_v = bass.AP(
                            tensor=xv_ap.tensor,
                            offset=b * K * N + (4 * q) * N,
                            ap=[[QW, 128], [1, QW]],
                        )
                        vt = vp.tile([128, QW], f16)
                        nc.sync.dma_start(out=vt[:], in_=src_v)

                        et = ep.tile([128, QW], f32r)
                        nc.scalar.activation(
                            et[:], ht[:], Exp,
                            accum_out=stats_t[:, b * 5 + q:b * 5 + q + 1],
                        )
                        mt = mp.tile([128, QW], bf16)
                        nc.vector.tensor_tensor(
                            out=mt[:], in0=et[:], in1=vt[:],
                            op=mybir.AluOpType.mult,
                        )

                        tb = b * 16 + 4 * q
                        statr = mq_r[:, 128 - tb:256 - tb]
                        last = b == BPC - 1 and q == NQ - 1
                        for j in range(4):
                            nc.tensor.matmul(
                                out=psE[:, j * 512:(j + 1) * 512],
                                lhsT=statr, rhs=et[:, j * 512:(j + 1) * 512],
                                start=j not in touched, stop=last,
                            )
                            touched.add(j)
                        mcol = BPC * 5 + b * 5 + q
                        if (b + q) % 2 == 0:
                            nc.scalar.activation(
                                mt[:], mt[:], Ident,
                                accum_out=stats_t[:, mcol:mcol + 1],
                            )
                        else:
                            nc.vector.reduce_sum(
                                stats_t[:, mcol:mcol + 1], mt[:],
                                axis=mybir.AxisListType.X,
                            )

                # single channels c=16 (heatmap) / c=33 (uncertainty) first:
                # their small serial PSUM chain completes early, hidden
                # under the DMA ramp; the end of the stream is then pure
                # deep-pipelined quad work.
                for b in range(BPC):
                    src_hs = bass.AP(
                        tensor=x_ap.tensor,
                        offset=b * K * N + 16 * N,
                        ap=[[SW, 128], [1, SW]],
                    )
                    hs = hp.tile([128, SW], f32r, tag="hs")
                    nc.sync.dma_start(out=hs[:], in_=src_hs)
                    src_vs = bass.AP(
                        tensor=xv_ap.tensor,
                        offset=b * K * N + 16 * N,
                        ap=[[SW, 128], [1, SW]],
                    )
                    vs = vp.tile([128, SW], f16, tag="vs")
                    nc.sync.dma_start(out=vs[:], in_=src_vs)

                    es = ep.tile([128, SW], f32r, tag="es")
                    nc.scalar.activation(
                        es[:], hs[:], Exp,
                        accum_out=stats_t[:, b * 5 + 4:b * 5 + 5],
                    )
                    msi = mp.tile([128, SW], bf16, tag="msi")
                    nc.vector.tensor_tensor(
                        out=msi[:], in0=es[:], in1=vs[:], op=mybir.AluOpType.mult,
                    )
                    stats_s = ms_r[:, 128 - b:256 - b]
                    nc.tensor.matmul(out=psEs[:, :], lhsT=stats_s, rhs=es[:],
                                     start=(b == 0), stop=(b == BPC - 1))
                    mscol = BPC * 5 + b * 5 + 4
                    nc.vector.reduce_sum(
                        stats_t[:, mscol:mscol + 1], msi[:],
                        axis=mybir.AxisListType.X,
                    )

                # T2 dots against the position ramp, straight off PSUM
                cel = dr.tile([128, 2048], f32)
                nc.vector.tensor_tensor(out=cel[:], in0=psE[:], in1=lf[:],
                                        op=mybir.AluOpType.mult)
                nc.vector.reduce_sum(aux_t[:, 0:1], cel[:],
                                     axis=mybir.AxisListType.X)
                ces = dr.tile([8, 512], f32, tag="ces")
                nc.vector.tensor_tensor(out=ces[:], in0=psEs[0:8, :],
                                        in1=lf[0:8, 0:512],
                                        op=mybir.AluOpType.mult)
                nc.vector.reduce_sum(aux_t[0:8, 2:3], ces[:],
                                     axis=mybir.AxisListType.X)

            nc.sync.dma_start(out=stats_d[:], in_=stats_t[:])
            nc.sync.dma_start(out=aux_d[:], in_=aux_t[:])

    nc.compile()
    return nc


def _masters():
    mq = np.zeros((128, 256), np.float32)
    for p in range(128):
        mq[p, 128 + p // 32] = 1.0
    ms = np.zeros((128, 256), np.float32)
    ms[:, 128] = 1.0
    return {"mq_r": mq, "ms_r": ms}


def _split_inputs(x: np.ndarray):
    """Per-core h (f32, channels 0..16) and v (fp16, channels 17..33).
    fp16 v halves the v-side HBM traffic; vi tolerates it easily.
    astype/copy release the GIL, so thread across cores and halves."""
    from concurrent.futures import ThreadPoolExecutor

    xr = x.reshape(B, C, N)

    def mk(i):
        sl = xr[i * BPC:(i + 1) * BPC]
        return (np.ascontiguousarray(sl[:, :K]),
                sl[:, K:].astype(np.float16))

    with ThreadPoolExecutor(max_workers=NCORES) as ex:
        return list(ex.map(mk, range(NCORES)))


def _run_device(x: np.ndarray, reps: int = 1, trace: bool = False,
                tmpdir: str | None = None):
    from concourse.bass_utils import run_bass_kernel_spmd

    if reps not in _cache:
        _cache[reps] = _build(reps)
    nc = _cache[reps]
    m = _masters()
    in_maps = [
        {"x": xh, "xv": xv, **m}
        for xh, xv in _split_inputs(x)
    ]
    return run_bass_kernel_spmd(nc, in_maps, list(range(NCORES)),
                                trace=trace, tmpdir=tmpdir)


def _finish(results) -> np.ndarray:
    out = np.empty((B, K, 3), np.float32)
    s32 = np.arange(32, dtype=np.float64)    # spatial block within a quad row
    p128 = np.arange(128, dtype=np.float64)  # spatial block for singles
    for i in range(NCORES):
        r = results[i]
        stats = r["stats"].astype(np.float64)   # [128, 80]
        aux = r["aux"].astype(np.float64)       # [128, 4]
        for b in range(BPC):
            for c in range(K):
                if c < 16:
                    q, cc = c // 4, c % 4
                    rv = stats[cc * 32:(cc + 1) * 32, b * 5 + q]  # [32]
                    s0 = rv.sum()
                    t1 = (2048.0 * s32 * rv).sum()
                    t2 = aux[b * 16 + c, 0]
                    s2 = stats[cc * 32:(cc + 1) * 32, BPC * 5 + b * 5 + q].sum()
                else:
                    rv = stats[:, b * 5 + 4]
                    s0 = rv.sum()
                    t1 = (512.0 * p128 * rv).sum()
                    t2 = aux[b, 2]
                    s2 = stats[:, BPC * 5 + b * 5 + 4].sum()
                ki = np.round((t1 + t2) / s0)
                out[i * BPC + b, c, 0] = np.mod(ki, W) / W * IMG_W
                out[i * BPC + b, c, 1] = np.floor(ki / W) / H * IMG_H
                out[i * BPC + b, c, 2] = s2 / s0
    return out


def kernel(x: np.ndarray) -> np.ndarray:
    res = _run_device(np.asarray(x), reps=1)
    return _finish(res.results)
